# revision 46
# baseline (speedup 1.0000x reference)
"""Tensor-parallel LlamaDecoderLayer forward on 8 Trainium2 NeuronCores.

Sharding (per the TP hint):
- attention: 4 q-heads + 1 kv-head per core (GQA groups align with cores);
  o-proj row-sharded; partial outputs reduce-scattered over tokens (bf16 wire)
- norm2 computed on each core's 256-token shard; normalized activations
  all-gathered (bf16)
- MLP: gate/up column-sharded / down row-sharded over the intermediate dim
  (zero-padded 11008 -> 11264 so every core gets 1408 = 11*128);
  down partials reduce-scattered over tokens; final residual added on the
  token shard and returned per-core, assembled on host.

Layout: activations are kept feature-major ([feature, token], feature on
SBUF partitions) so every weight matrix loads as lhsT in its natural layout.
RMSNorm is folded into the matmul epilogue (scale columns of the product by
the per-token rms), softmax runs in transposed [k_tok, q_tok] layout without
max-subtraction (scores are bounded; fp32 exp cannot overflow), and
denominators come from ones-vector matmuls (partition reduction on the PE).

Performance structure:
- all matmuls run at 1 PE cycle/row: bf16 for qkv/o-proj/MLP, fp32r (raw
  fp32 streaming) for attention scores/AV and the sum-of-squares reductions
- wqkv is bf16 and SBUF-resident (loaded once); x streams via gpsimd
  casting DMAs (f32 DRAM -> bf16 SBUF); wo/wgu/wdn are packed host-side
  into contiguous per-slab blocks so weight DMA bursts are >= 1KB
- RoPE is applied per token chunk inside phase A (overlaps next chunk's PE)
- attention runs two heads in flight so one stream's exp hides under the
  other's matmuls; per-stream denominators use separate PSUM banks
- RS1 and RS2 split in feature quarters: early quarters reduce while
  later o-proj/down-proj quarters still compute, and only a quarter of
  the dependent work waits for the last piece; the MLP AllGather stays
  single (splitting it can't overlap enough compute to amortize the
  extra launch)
- PSUM->bf16 spills run on DVE; write-back DMAs issue from the Activation
  HWDGE queue so weight prefetches on the SP queue never starve the PE
"""

import numpy as np
import ml_dtypes

import concourse.bacc as bacc
import concourse.bass as bass
import concourse.bass_isa as bass_isa
import concourse.mybir as mybir
import concourse.tile as tile
from concourse.bass_utils import run_bass_kernel_spmd
from concourse.masks import make_identity

AF = mybir.ActivationFunctionType
ALU = mybir.AluOpType
DT = mybir.dt
BF16 = ml_dtypes.bfloat16

FULL_CFG = dict(H=4096, S=1024, B=2, NQH=4, D=128, IC=1408, NC=8, EPS=1e-5)


def build_nc(cfg):
    H, S, B = cfg["H"], cfg["S"], cfg["B"]
    NQH, D, IC = cfg["NQH"], cfg["D"], cfg["IC"]
    NCORES, EPS = cfg["NC"], cfg["EPS"]
    T = B * S
    KT = H // 128          # hidden-dim k tiles
    FQK = NQH + 1          # q tiles + 1 k tile (feature-major outputs)
    NF = FQK + 1           # + v tile -> qkv feature tiles
    KI = IC // 128         # intermediate k tiles (per-core shard)
    TS = T // NCORES       # token shard (reduce-scatter granularity)
    TQ = min(512, S)       # attention query chunk
    NQC = S // TQ
    DIAG = TQ // 128       # diagonal (masked) kt blocks per query chunk
    TCH = min(512, T)      # matmul token chunk
    NTC = T // TCH
    NSH = TCH // TS if TCH >= TS else 1   # shard blocks per token chunk
    MH = H // 128          # output feature tiles
    SB = S // 128          # seq kt blocks per batch
    sm_scale = float(1.0 / np.sqrt(D))
    f32, bf16 = DT.float32, DT.bfloat16

    nc = bacc.Bacc("TRN2", target_bir_lowering=False, debug=False,
                   num_devices=NCORES)

    f32r = DT.float32r

    def mmr(out, lhsT, rhs, **kw):
        # fp32 operands streamed in raw mode: 1 PE cycle/row at free dim
        # >= 256 (vs 4 for decomposed fp32), bf16-class operand precision
        nc.tensor.matmul(out, lhsT.bitcast(f32r), rhs.bitcast(f32r), **kw)

    xt = nc.dram_tensor("xt", [H, T], f32, kind="ExternalInput")
    xs = nc.dram_tensor("xs", [H, TS], f32, kind="ExternalInput")
    wqkv = nc.dram_tensor("wqkv", [H, NF * 128], bf16, kind="ExternalInput")
    # weight slabs packed host-side: each [128, K*128] slab is one
    # contiguous DRAM block (256B-segment reads halve DMA bus efficiency)
    wo = nc.dram_tensor("wo", [MH, 128, NQH * 128], bf16,
                        kind="ExternalInput")
    wgu = nc.dram_tensor("wgu", [2 * KI, 128, KT * 128], bf16,
                         kind="ExternalInput")
    wdn = nc.dram_tensor("wdn", [MH, 128, KI * 128], bf16,
                         kind="ExternalInput")
    cs = nc.dram_tensor("cs", [D, T], f32, kind="ExternalInput")
    sn = nc.dram_tensor("sn", [D, T], f32, kind="ExternalInput")
    msk = nc.dram_tensor("msk", [DIAG, 128, TQ], f32, kind="ExternalInput")
    out_t = nc.dram_tensor("out_t", [H, TS], f32, kind="ExternalOutput")

    wqkv_r = wqkv.ap().rearrange("(ko p) f -> p ko f", p=128)
    rg = [list(range(NCORES))]

    with tile.TileContext(nc, num_cores=NCORES) as tc:
        with (
            tc.tile_pool(name="misc", bufs=1) as miscp,
            tc.tile_pool(name="small", bufs=2) as smallp,
            tc.tile_pool(name="dram", bufs=1, space="DRAM") as dramp,
            tc.tile_pool(name="ps", bufs=1, space="PSUM") as psp,
        ):
            ones_f = miscp.tile([128, 1], f32, tag="ones_f")
            nc.gpsimd.memset(ones_f, 1.0)
            ones_col = miscp.tile([128, 1], f32, tag="ones_col")
            nc.scalar.copy(ones_col.bitcast(f32r), ones_f)
            eps_col = miscp.tile([128, 1], f32, tag="eps_col")
            nc.gpsimd.memset(eps_col, EPS)

            # reduce-scatters split along features so early parts overlap
            # with compute; the AllGather stays whole (nothing to overlap)
            HQ = H // 4
            bounce1q = [dramp.tile([NCORES, HQ, TS], bf16,
                                   tag=f"bounce1q{q}", name=f"bounce1q{q}")
                        for q in range(4)]
            rs1q = [dramp.tile([HQ, TS], bf16, tag=f"rs1q{q}",
                               name=f"rs1q{q}")
                    for q in range(4)]
            ag_in = dramp.tile([H, TS], bf16, tag="ag_in")
            ag_out = dramp.tile([NCORES, H, TS], bf16, tag="ag_out",
                                addr_space="Shared")
            bounce2q = [dramp.tile([NCORES, HQ, TS], bf16,
                                   tag=f"bounce2q{q}", name=f"bounce2q{q}")
                        for q in range(4)]
            rs2q = [dramp.tile([HQ, TS], bf16, tag=f"rs2q{q}",
                               name=f"rs2q{q}")
                    for q in range(4)]

            # ============ scope 1: qkv + attention + o-proj ============
            with (
                tc.tile_pool(name="bigAB", bufs=1) as bigp,
                tc.tile_pool(name="strAB", bufs=4) as strp,
                tc.tile_pool(name="tmpAB", bufs=3) as tmpp,
            ):
                ident = bigp.tile([128, 128], f32, tag="ident")
                make_identity(nc, ident)
                cs_sb = bigp.tile([128, T], f32, tag="cs")
                sn_sb = bigp.tile([128, T], f32, tag="sn")
                for i in range(T // 512):
                    nc.sync.dma_start(cs_sb[:, i * 512:(i + 1) * 512],
                                      cs.ap()[:, i * 512:(i + 1) * 512])
                    nc.sync.dma_start(sn_sb[:, i * 512:(i + 1) * 512],
                                      sn.ap()[:, i * 512:(i + 1) * 512])
                msk_sb = bigp.tile([128, DIAG, TQ], f32, tag="msk")
                for j in range(DIAG):
                    nc.sync.dma_start(msk_sb[:, j, :], msk.ap()[j])

                qk_sb = bigp.tile([128, FQK, T], f32, tag="qk")
                v_tok = bigp.tile([128, T // 128, 128], f32, tag="vtok")
                attn_sb = bigp.tile([128, NQH, T], bf16, tag="attn")

                # wqkv (bf16) resident in SBUF: loaded once, reused by all
                # token chunks (re-streaming fp32 weights made phase A
                # DMA-bound)
                w_sb = bigp.tile([128, KT, NF * 128], bf16, tag="w_sb")
                for k in range(KT):
                    nc.sync.dma_start(w_sb[:, k, :], wqkv_r[:, k, :])

                # ---- phase A: rmsnorm-folded qkv ----
                for tci in range(NTC):
                    t0 = tci * TCH
                    ps_qkv = [psp.tile([128, TCH], f32, tag="acc", bufs=6,
                                       name=f"ps_qkv{f}")
                              for f in range(NF)]
                    sq_acc = strp.tile([128, 2, TCH], f32, tag="sq_acc",
                                       bufs=1)
                    for kp in range(KT // 2):
                        xtile = strp.tile([128, 2, TCH], bf16, tag="x_in",
                                          bufs=3)
                        nc.gpsimd.dma_start(
                            xtile,
                            xt.ap()[kp * 256:(kp + 1) * 256,
                                    t0:t0 + TCH].rearrange(
                                        "(g p) t -> p g t", p=128))
                        sq = strp.tile([128, 2, TCH], f32, tag="sq", bufs=2)
                        nc.scalar.square(sq, xtile)
                        if kp == 0:
                            nc.vector.tensor_scalar_mul(sq_acc, sq, 1.0)
                        else:
                            nc.vector.tensor_tensor(sq_acc, sq_acc, sq,
                                                    ALU.add)
                        for g in range(2):
                            k = 2 * kp + g
                            for f in range(NF):
                                nc.tensor.matmul(
                                    ps_qkv[f],
                                    w_sb[:, k, f * 128:(f + 1) * 128],
                                    xtile[:, g, :], start=(k == 0),
                                    stop=(k == KT - 1),
                                    skip_group_check=True)
                    ssf = smallp.tile([128, TCH], f32, tag="ssf", bufs=1)
                    nc.vector.tensor_tensor(ssf, sq_acc[:, 0, :],
                                            sq_acc[:, 1, :], ALU.add)
                    ssr = smallp.tile([128, TCH], f32, tag="ssr", bufs=1)
                    nc.gpsimd.partition_all_reduce(ssr, ssf, 128,
                                                   bass_isa.ReduceOp.add)
                    ms = smallp.tile([128, TCH], f32, tag="ms", bufs=1)
                    nc.scalar.activation(ms, ssr, AF.Sqrt,
                                         bias=eps_col, scale=1.0 / H)
                    rb = smallp.tile([128, TCH], f32, tag="rb_a")
                    nc.vector.reciprocal(rb, ms)
                    for f in range(FQK):
                        nc.vector.tensor_tensor(
                            qk_sb[:, f, t0:t0 + TCH].bitcast(f32r),
                            ps_qkv[f], rb, ALU.mult)
                    v_t = tmpp.tile([128, TCH], f32, tag="v_t", bufs=2)
                    nc.vector.tensor_tensor(v_t, ps_qkv[NF - 1], rb, ALU.mult)
                    for j in range(TCH // 128):
                        ps_tr = psp.tile([128, 128], f32, tag="tr", bufs=1)
                        nc.tensor.transpose(
                            ps_tr, v_t[:, j * 128:(j + 1) * 128], ident)
                        nc.scalar.copy(
                            v_tok[:, (t0 // 128) + j, :].bitcast(f32r), ps_tr)

                    # ---- RoPE on this chunk (overlaps next chunk's PE) ----
                    hd = D // 2
                    for f in range(FQK):
                        qs = qk_sb[:, f, t0:t0 + TCH]
                        tmp = tmpp.tile([128, TCH], f32, tag="rope_tmp",
                                        bufs=2)
                        nc.scalar.copy(tmp[0:hd, :],
                                       qk_sb[hd:D, f, t0:t0 + TCH])
                        nc.scalar.copy(tmp[hd:D, :],
                                       qk_sb[0:hd, f, t0:t0 + TCH])
                        nc.vector.tensor_tensor(tmp, tmp,
                                                sn_sb[:, t0:t0 + TCH],
                                                ALU.mult)
                        nc.vector.tensor_tensor(qs.bitcast(f32r), qs,
                                                cs_sb[:, t0:t0 + TCH],
                                                ALU.mult)
                        nc.vector.tensor_tensor(qs.bitcast(f32r), qs, tmp,
                                                ALU.add)

                # ---- phase B: attention (two heads in flight so one
                # stream's exp hides under the other's matmuls) ----
                for b in range(B):
                    for qc in range(NQC):
                        qt0 = qc * TQ
                        nkt = (qt0 + TQ) // 128
                        diag0 = nkt - DIAG
                        for hp in range(NQH // 2):
                            hs = (2 * hp, 2 * hp + 1)
                            ps_o = [psp.tile([128, TQ], f32, tag="acc",
                                             bufs=6, name=f"ps_o{i}")
                                    for i in range(2)]
                            dens = [psp.tile([1, TQ], f32, tag="ss", bufs=1,
                                             name="den0"),
                                    psp.tile([1, TQ], f32, tag="tr", bufs=1,
                                             name="den1")]
                            for kt in range(nkt):
                                es = []
                                for i, h in enumerate(hs):
                                    ps_s = psp.tile([128, TQ], f32,
                                                    tag="acc", bufs=6)
                                    mmr(ps_s,
                                        qk_sb[:, NQH, b * S + kt * 128:
                                              b * S + (kt + 1) * 128],
                                        qk_sb[:, h,
                                              b * S + qt0:b * S + qt0 + TQ],
                                        start=True, stop=True,
                                        skip_group_check=True)
                                    e_sb = tmpp.tile([128, TQ], f32,
                                                     tag="e_sb", bufs=4)
                                    nc.scalar.activation(
                                        e_sb.bitcast(f32r), ps_s, AF.Exp,
                                        scale=sm_scale)
                                    if kt >= diag0:
                                        nc.vector.tensor_tensor(
                                            e_sb.bitcast(f32r), e_sb,
                                            msk_sb[:, kt - diag0, :],
                                            ALU.mult)
                                    es.append(e_sb)
                                for i in range(2):
                                    mmr(ps_o[i], v_tok[:, b * SB + kt, :],
                                        es[i], start=(kt == 0),
                                        stop=(kt == nkt - 1),
                                        skip_group_check=True)
                                    mmr(dens[i], ones_col, es[i],
                                        start=(kt == 0),
                                        stop=(kt == nkt - 1),
                                        skip_group_check=True)
                            for i, h in enumerate(hs):
                                rden = smallp.tile([1, TQ], f32, tag="rden")
                                nc.vector.reciprocal(rden, dens[i])
                                rbd = smallp.tile([128, TQ], f32, tag="rbd")
                                nc.gpsimd.partition_broadcast(rbd, rden, 128)
                                nc.vector.tensor_tensor(
                                    attn_sb[:, h,
                                            b * S + qt0:b * S + qt0 + TQ],
                                    ps_o[i], rbd, ALU.mult)

                # ---- phase C1: o-proj + 4-way split RS1 (early quarters
                # reduce while later quarters still compute) ----
                qM1 = MH // 4
                for m in range(MH):
                    wos = strp.tile([128, NQH * 128], bf16, tag="wo_slab",
                                    bufs=6)
                    nc.sync.dma_start(wos, wo.ap()[m])
                    tgt = bounce1q[m // qM1]
                    mrow = (m % qM1) * 128
                    for tci in range(NTC):
                        ps = psp.tile([128, TCH], f32, tag="acc", bufs=6)
                        for kh in range(NQH):
                            nc.tensor.matmul(
                                ps, wos[:, kh * 128:(kh + 1) * 128],
                                attn_sb[:, kh, tci * TCH:(tci + 1) * TCH],
                                start=(kh == 0), stop=(kh == NQH - 1))
                        ob = tmpp.tile([128, TCH], bf16, tag="o_bf", bufs=4)
                        nc.vector.tensor_scalar_mul(ob, ps, 1.0)
                        nc.scalar.dma_start(
                            tgt[tci * NSH:(tci + 1) * NSH,
                                mrow:mrow + 128, :].rearrange(
                                    "s p t -> p s t"),
                            ob.rearrange("p (s t) -> p s t", s=NSH))
                    if (m + 1) % qM1 == 0:
                        q = m // qM1
                        nc.gpsimd.collective_compute(
                            "ReduceScatter", ALU.add, replica_groups=rg,
                            ins=[bounce1q[q].opt()], outs=[rs1q[q].opt()])

            # ============ scope 2: norm2 + AG + MLP + final ============
            with (
                tc.tile_pool(name="bigCD", bufs=1) as bigp2,
                tc.tile_pool(name="strCD", bufs=4) as strp2,
                tc.tile_pool(name="tmpCD", bufs=2) as tmpp2,
            ):
                h_sh = bigp2.tile([128, KT, TS], f32, tag="h_sh")
                act_sb = bigp2.tile([128, KI, T], bf16, tag="act")

                qK1 = KT // 4
                ps_ss2 = psp.tile([1, TS], f32, bufs=1, tag="ss")
                for kp in range(KT // 2):
                    k0 = 2 * kp
                    srcb = rs1q[k0 // qK1]
                    krow = (k0 % qK1) * 128
                    rt = strp2.tile([128, 2, TS], bf16, tag="rs1_t", bufs=3)
                    nc.sync.dma_start(
                        rt, srcb[krow:krow + 256, :].rearrange(
                            "(g p) t -> p g t", p=128))
                    xst = strp2.tile([128, 2, TS], f32, tag="xs_t", bufs=3)
                    nc.sync.dma_start(
                        xst, xs.ap()[k0 * 128:(k0 + 2) * 128, :].rearrange(
                            "(g p) t -> p g t", p=128))
                    nc.vector.tensor_tensor(
                        h_sh[:, k0:k0 + 2, :], rt, xst, ALU.add)
                    sq2 = strp2.tile([128, 2, TS], f32, tag="sq2", bufs=3)
                    nc.scalar.square(sq2.bitcast(f32r), h_sh[:, k0:k0 + 2, :])
                    for g in range(2):
                        k = k0 + g
                        mmr(ps_ss2, ones_col, sq2[:, g, :],
                            start=(k == 0), stop=(k == KT - 1),
                            skip_group_check=True)
                ms2 = smallp.tile([1, TS], f32, tag="ms2")
                nc.scalar.activation(ms2, ps_ss2, AF.Sqrt,
                                     bias=eps_col[0:1, :], scale=1.0 / H)
                rr2 = smallp.tile([1, TS], f32, tag="rr2")
                nc.vector.reciprocal(rr2, ms2)
                rb2 = smallp.tile([128, TS], f32, tag="rb2")
                nc.gpsimd.partition_broadcast(rb2, rr2, 128)
                for k in range(KT):
                    mib = tmpp2.tile([128, TS], bf16, tag="mib", bufs=4)
                    nc.vector.tensor_tensor(mib, h_sh[:, k, :], rb2, ALU.mult)
                    nc.scalar.dma_start(ag_in[k * 128:(k + 1) * 128, :], mib)
                nc.gpsimd.collective_compute(
                    "AllGather", ALU.bypass, replica_groups=rg,
                    ins=[ag_in.opt()], outs=[ag_out.opt()])

                # ---- phase D: gate/up ----
                for tci in range(NTC):
                    t0 = tci * TCH
                    mi = strp2.tile([128, KT, TCH], bf16, tag="mi", bufs=1)
                    for k in range(KT):
                        src = ag_out[tci * NSH:(tci + 1) * NSH,
                                     k * 128:(k + 1) * 128, :]
                        nc.sync.dma_start(
                            mi[:, k, :].rearrange("p (b t) -> p b t", b=NSH),
                            src.rearrange("b p t -> p b t"))
                    for fi in range(KI):
                        wg = strp2.tile([128, KT * 128], bf16, tag="wg_slab",
                                        bufs=2)
                        nc.sync.dma_start(wg, wgu.ap()[fi])
                        wu = strp2.tile([128, KT * 128], bf16, tag="wu_slab",
                                        bufs=2)
                        nc.sync.dma_start(wu, wgu.ap()[KI + fi])
                        ps_g = psp.tile([128, TCH], f32, tag="acc", bufs=6)
                        ps_u = psp.tile([128, TCH], f32, tag="acc", bufs=6)
                        for k in range(KT):
                            nc.tensor.matmul(ps_g,
                                             wg[:, k * 128:(k + 1) * 128],
                                             mi[:, k, :], start=(k == 0),
                                             stop=(k == KT - 1),
                                             skip_group_check=True)
                        for k in range(KT):
                            nc.tensor.matmul(ps_u,
                                             wu[:, k * 128:(k + 1) * 128],
                                             mi[:, k, :], start=(k == 0),
                                             stop=(k == KT - 1),
                                             skip_group_check=True)
                        sg = tmpp2.tile([128, TCH], f32, tag="sg", bufs=2)
                        nc.scalar.activation(sg, ps_g, AF.Silu)
                        nc.vector.tensor_tensor(act_sb[:, fi, t0:t0 + TCH],
                                                sg, ps_u, ALU.mult)

                # ---- phase D2: down-proj + 4-way split RS2 + residual ----
                qM = MH // 4
                for m in range(MH):
                    wds = strp2.tile([128, KI * 128], bf16, tag="wd_slab",
                                     bufs=3)
                    nc.sync.dma_start(wds, wdn.ap()[m])
                    tgt = bounce2q[m // qM]
                    mrow = (m % qM) * 128
                    for tci in range(NTC):
                        ps = psp.tile([128, TCH], f32, tag="acc", bufs=6)
                        for k in range(KI):
                            nc.tensor.matmul(
                                ps, wds[:, k * 128:(k + 1) * 128],
                                act_sb[:, k, tci * TCH:(tci + 1) * TCH],
                                start=(k == 0), stop=(k == KI - 1))
                        db = tmpp2.tile([128, TCH], bf16, tag="d_bf", bufs=4)
                        nc.vector.tensor_scalar_mul(db, ps, 1.0)
                        nc.scalar.dma_start(
                            tgt[tci * NSH:(tci + 1) * NSH,
                                mrow:mrow + 128, :].rearrange(
                                    "s p t -> p s t"),
                            db.rearrange("p (s t) -> p s t", s=NSH))
                    if (m + 1) % qM == 0:
                        q = m // qM
                        nc.gpsimd.collective_compute(
                            "ReduceScatter", ALU.add, replica_groups=rg,
                            ins=[bounce2q[q].opt()], outs=[rs2q[q].opt()])

                qK = KT // 4
                for k in range(KT):
                    r2t = strp2.tile([128, TS], bf16, tag="rs2_t", bufs=4)
                    nc.sync.dma_start(
                        r2t, rs2q[k // qK][(k % qK) * 128:(k % qK) * 128 + 128, :])
                    ot = tmpp2.tile([128, TS], f32, tag="out_t", bufs=4)
                    nc.vector.tensor_tensor(ot, r2t, h_sh[:, k, :], ALU.add)
                    nc.scalar.dma_start(out_t.ap()[k * 128:(k + 1) * 128, :],
                                        ot)

    nc.compile()
    return nc


def prepare_inputs(inputs, cfg):
    """Full np inputs -> per-core in_maps (host-side sharding/prep)."""
    H, S, B = cfg["H"], cfg["S"], cfg["B"]
    NQH, D, IC = cfg["NQH"], cfg["D"], cfg["IC"]
    NCORES = cfg["NC"]
    T = B * S
    TS = T // NCORES
    TQ = min(512, S)
    DIAG = TQ // 128
    f4 = np.float32

    x = np.asarray(inputs["x"], f4).reshape(T, H)
    xt = np.ascontiguousarray(x.T)                      # [H, T]
    cos = np.asarray(inputs["cos"], f4)                 # [S, D]
    sin = np.asarray(inputs["sin"], f4)
    cs = np.ascontiguousarray(np.tile(cos.T, (1, B)))   # [D, T]
    sn_s = sin.T.copy()
    sn_s[: D // 2] *= -1.0                              # sign-folded rot_half
    sn = np.ascontiguousarray(np.tile(sn_s, (1, B)))
    anw = np.asarray(inputs["attn_norm_w"], f4)
    fnw = np.asarray(inputs["ffn_norm_w"], f4)
    wq = np.asarray(inputs["wq"], f4) * anw[:, None]
    wk = np.asarray(inputs["wk"], f4) * anw[:, None]
    wv = np.asarray(inputs["wv"], f4) * anw[:, None]
    # wqkv ships bf16 (kernel keeps it SBUF-resident)
    wo = np.asarray(inputs["wo"], f4)
    wg = np.asarray(inputs["w_gate"], f4) * fnw[:, None]
    wu = np.asarray(inputs["w_up"], f4) * fnw[:, None]
    wd = np.asarray(inputs["w_down"], f4)
    I_full = wg.shape[1]
    I_pad = NCORES * IC
    if I_pad > I_full:
        pad = I_pad - I_full
        wg = np.pad(wg, ((0, 0), (0, pad)))
        wu = np.pad(wu, ((0, 0), (0, pad)))
        wd = np.pad(wd, ((0, pad), (0, 0)))

    # causal masks, transposed layout: keep (j*128 + kt) <= q
    kt_i = np.arange(128)[:, None]
    q_i = np.arange(TQ)[None, :]
    msk = np.stack([(j * 128 + kt_i <= q_i).astype(f4) for j in range(DIAG)])

    def pack_slabs(w):
        # [K*128, ncols] -> [ncols//128, 128, K*128]: one contiguous
        # DRAM block per 128-column output slab
        K = w.shape[0] // 128
        nc_ = w.shape[1] // 128
        return np.ascontiguousarray(
            w.reshape(K, 128, nc_, 128).transpose(2, 1, 0, 3)
            .reshape(nc_, 128, K * 128))

    qd, kvd = NQH * D, D  # per-core q cols, kv cols
    in_maps = []
    for c in range(NCORES):
        wqkv = np.concatenate([
            wq[:, c * qd:(c + 1) * qd],
            wk[:, c * kvd:(c + 1) * kvd],
            wv[:, c * kvd:(c + 1) * kvd]], axis=1)
        wgu = np.concatenate([
            pack_slabs(wg[:, c * IC:(c + 1) * IC]),
            pack_slabs(wu[:, c * IC:(c + 1) * IC])], axis=0).astype(BF16)
        in_maps.append({
            "xt": xt,
            "xs": np.ascontiguousarray(xt[:, c * TS:(c + 1) * TS]),
            "wqkv": np.ascontiguousarray(wqkv.astype(BF16)),
            "wo": pack_slabs(wo[c * qd:(c + 1) * qd, :]).astype(BF16),
            "wgu": np.ascontiguousarray(wgu),
            "wdn": pack_slabs(wd[c * IC:(c + 1) * IC, :]).astype(BF16),
            "cs": cs, "sn": sn, "msk": msk,
        })
    return in_maps


def assemble_output(results, cfg):
    H, S, B, NCORES = cfg["H"], cfg["S"], cfg["B"], cfg["NC"]
    full_t = np.concatenate([r["out_t"] for r in results], axis=1)  # [H, T]
    return np.ascontiguousarray(full_t.T).reshape(B, S, H)


_NC_CACHE = {}


def _get_nc(cfg_key, cfg):
    if cfg_key not in _NC_CACHE:
        _NC_CACHE[cfg_key] = build_nc(cfg)
    return _NC_CACHE[cfg_key]


def run(inputs, cfg, **kwargs):
    nc = _get_nc(tuple(sorted(cfg.items())), cfg)
    in_maps = prepare_inputs(inputs, cfg)
    res = run_bass_kernel_spmd(nc, in_maps,
                               core_ids=list(range(cfg["NC"])), **kwargs)
    return assemble_output(res.results, cfg), res


def kernel(**inputs) -> np.ndarray:
    out, _ = run(inputs, FULL_CFG)
    return out



# revision 49
# speedup vs baseline: 1.1691x; 1.1691x over previous
"""Tensor-parallel LlamaDecoderLayer forward on 8 Trainium2 NeuronCores.

Sharding (per the TP hint):
- attention: 4 q-heads + 1 kv-head per core (GQA groups align with cores);
  o-proj row-sharded; partial outputs reduce-scattered over tokens (bf16 wire)
- norm2 computed on each core's 256-token shard; normalized activations
  all-gathered (bf16)
- MLP: gate/up column-sharded / down row-sharded over the intermediate dim
  (zero-padded 11008 -> 11264 so every core gets 1408 = 11*128);
  down partials reduce-scattered over tokens; final residual added on the
  token shard and returned per-core, assembled on host.

Layout: activations are kept feature-major ([feature, token], feature on
SBUF partitions) so every weight matrix loads as lhsT in its natural layout.
RMSNorm is folded into the matmul epilogue (scale columns of the product by
the per-token rms), softmax runs in transposed [k_tok, q_tok] layout without
max-subtraction (scores are bounded; fp32 exp cannot overflow), and
denominators come from ones-vector matmuls (partition reduction on the PE).

Performance structure:
- all matmuls run at 1 PE cycle/row: bf16 for qkv/o-proj/MLP, fp32r (raw
  fp32 streaming) for attention scores/AV and norm2's sum-of-squares;
  phase A's rmsnorm reduction runs off the PE entirely (DVE accumulates
  squares across k-tiles, one gpsimd partition_all_reduce per chunk whose
  all-partition output also replaces the reciprocal broadcast)
- wqkv is bf16 and SBUF-resident (loaded once); x streams via gpsimd
  casting DMAs (f32 DRAM -> bf16 SBUF); wo/wgu/wdn are packed host-side
  into contiguous per-slab blocks so weight DMA bursts are >= 1KB
- RoPE is applied per token chunk inside phase A (overlaps next chunk's PE)
- attention runs two heads in flight so one stream's exp hides under the
  other's matmuls; per-stream denominators use separate PSUM banks
- RS1 and RS2 split in feature quarters: early quarters reduce while
  later o-proj/down-proj quarters still compute, and only a quarter of
  the dependent work waits for the last piece; the MLP AllGather stays
  single (splitting it can't overlap enough compute to amortize the
  extra launch)
- PSUM->bf16 spills run on DVE; write-back DMAs issue from the Activation
  HWDGE queue so weight prefetches on the SP queue never starve the PE
"""

import numpy as np
import ml_dtypes

import concourse.bacc as bacc
import concourse.bass as bass
import concourse.bass_isa as bass_isa
import concourse.mybir as mybir
import concourse.tile as tile
from concourse.bass_utils import run_bass_kernel_spmd
from concourse.masks import make_identity

AF = mybir.ActivationFunctionType
ALU = mybir.AluOpType
DT = mybir.dt
BF16 = ml_dtypes.bfloat16

FULL_CFG = dict(H=4096, S=1024, B=2, NQH=4, D=128, IC=1408, NC=8, EPS=1e-5)


def build_nc(cfg):
    H, S, B = cfg["H"], cfg["S"], cfg["B"]
    NQH, D, IC = cfg["NQH"], cfg["D"], cfg["IC"]
    NCORES, EPS = cfg["NC"], cfg["EPS"]
    T = B * S
    KT = H // 128          # hidden-dim k tiles
    FQK = NQH + 1          # q tiles + 1 k tile (feature-major outputs)
    NF = FQK + 1           # + v tile -> qkv feature tiles
    KI = IC // 128         # intermediate k tiles (per-core shard)
    TS = T // NCORES       # token shard (reduce-scatter granularity)
    TQ = min(512, S)       # attention query chunk
    NQC = S // TQ
    DIAG = TQ // 128       # diagonal (masked) kt blocks per query chunk
    TCH = min(512, T)      # matmul token chunk
    NTC = T // TCH
    NSH = TCH // TS if TCH >= TS else 1   # shard blocks per token chunk
    MH = H // 128          # output feature tiles
    SB = S // 128          # seq kt blocks per batch
    sm_scale = float(1.0 / np.sqrt(D))
    f32, bf16 = DT.float32, DT.bfloat16

    nc = bacc.Bacc("TRN2", target_bir_lowering=False, debug=False,
                   num_devices=NCORES)

    f32r = DT.float32r

    def mmr(out, lhsT, rhs, **kw):
        # fp32 operands streamed in raw mode: 1 PE cycle/row at free dim
        # >= 256 (vs 4 for decomposed fp32), bf16-class operand precision
        nc.tensor.matmul(out, lhsT.bitcast(f32r), rhs.bitcast(f32r), **kw)

    xt = nc.dram_tensor("xt", [H, T], f32, kind="ExternalInput")
    xs = nc.dram_tensor("xs", [H, TS], f32, kind="ExternalInput")
    wqkv = nc.dram_tensor("wqkv", [H, NF * 128], bf16, kind="ExternalInput")
    # weight slabs packed host-side: each [128, K*128] slab is one
    # contiguous DRAM block (256B-segment reads halve DMA bus efficiency)
    wo = nc.dram_tensor("wo", [MH, 128, NQH * 128], bf16,
                        kind="ExternalInput")
    wgu = nc.dram_tensor("wgu", [2 * KI, 128, KT * 128], bf16,
                         kind="ExternalInput")
    wdn = nc.dram_tensor("wdn", [MH, 128, KI * 128], bf16,
                         kind="ExternalInput")
    cs = nc.dram_tensor("cs", [D, T], f32, kind="ExternalInput")
    sn = nc.dram_tensor("sn", [D, T], f32, kind="ExternalInput")
    msk = nc.dram_tensor("msk", [DIAG, 128, TQ], f32, kind="ExternalInput")
    out_t = nc.dram_tensor("out_t", [H, TS], f32, kind="ExternalOutput")

    wqkv_r = wqkv.ap().rearrange("(ko p) f -> p ko f", p=128)
    rg = [list(range(NCORES))]

    with tile.TileContext(nc, num_cores=NCORES) as tc:
        with (
            tc.tile_pool(name="misc", bufs=1) as miscp,
            tc.tile_pool(name="small", bufs=2) as smallp,
            tc.tile_pool(name="dram", bufs=1, space="DRAM") as dramp,
            tc.tile_pool(name="ps", bufs=1, space="PSUM") as psp,
        ):
            ones_f = miscp.tile([128, 1], f32, tag="ones_f")
            nc.gpsimd.memset(ones_f, 1.0)
            ones_col = miscp.tile([128, 1], f32, tag="ones_col")
            nc.scalar.copy(ones_col.bitcast(f32r), ones_f)
            eps_col = miscp.tile([128, 1], f32, tag="eps_col")
            nc.gpsimd.memset(eps_col, EPS)

            # reduce-scatters split along features so early parts overlap
            # with compute; the AllGather stays whole (nothing to overlap)
            HQ = H // 4
            bounce1q = [dramp.tile([NCORES, HQ, TS], bf16,
                                   tag=f"bounce1q{q}", name=f"bounce1q{q}")
                        for q in range(4)]
            rs1q = [dramp.tile([HQ, TS], bf16, tag=f"rs1q{q}",
                               name=f"rs1q{q}")
                    for q in range(4)]
            ag_in = dramp.tile([H, TS], bf16, tag="ag_in")
            ag_out = dramp.tile([NCORES, H, TS], bf16, tag="ag_out",
                                addr_space="Shared")
            bounce2q = [dramp.tile([NCORES, HQ, TS], bf16,
                                   tag=f"bounce2q{q}", name=f"bounce2q{q}")
                        for q in range(4)]
            rs2q = [dramp.tile([HQ, TS], bf16, tag=f"rs2q{q}",
                               name=f"rs2q{q}")
                    for q in range(4)]

            # ============ scope 1: qkv + attention + o-proj ============
            with (
                tc.tile_pool(name="bigAB", bufs=1) as bigp,
                tc.tile_pool(name="strAB", bufs=4) as strp,
                tc.tile_pool(name="tmpAB", bufs=3) as tmpp,
            ):
                ident = bigp.tile([128, 128], f32, tag="ident")
                make_identity(nc, ident)
                cs_sb = bigp.tile([128, T], f32, tag="cs")
                sn_sb = bigp.tile([128, T], f32, tag="sn")
                for i in range(T // 512):
                    nc.sync.dma_start(cs_sb[:, i * 512:(i + 1) * 512],
                                      cs.ap()[:, i * 512:(i + 1) * 512])
                    nc.sync.dma_start(sn_sb[:, i * 512:(i + 1) * 512],
                                      sn.ap()[:, i * 512:(i + 1) * 512])
                msk_sb = bigp.tile([128, DIAG, TQ], f32, tag="msk")
                for j in range(DIAG):
                    nc.sync.dma_start(msk_sb[:, j, :], msk.ap()[j])

                qk_sb = bigp.tile([128, FQK, T], f32, tag="qk")
                v_tok = bigp.tile([128, T // 128, 128], f32, tag="vtok")
                attn_sb = bigp.tile([128, NQH, T], bf16, tag="attn")

                # wqkv (bf16) resident in SBUF: loaded once, reused by all
                # token chunks (re-streaming fp32 weights made phase A
                # DMA-bound)
                w_sb = bigp.tile([128, KT, NF * 128], bf16, tag="w_sb")
                for k in range(KT):
                    nc.sync.dma_start(w_sb[:, k, :], wqkv_r[:, k, :])

                # ---- phase A: rmsnorm-folded qkv ----
                for tci in range(NTC):
                    t0 = tci * TCH
                    ps_qkv = [psp.tile([128, TCH], f32, tag="acc", bufs=6,
                                       name=f"ps_qkv{f}")
                              for f in range(NF)]
                    sq_acc = strp.tile([128, 2, TCH], f32, tag="sq_acc",
                                       bufs=1)
                    for kp in range(KT // 2):
                        xtile = strp.tile([128, 2, TCH], bf16, tag="x_in",
                                          bufs=3)
                        nc.gpsimd.dma_start(
                            xtile,
                            xt.ap()[kp * 256:(kp + 1) * 256,
                                    t0:t0 + TCH].rearrange(
                                        "(g p) t -> p g t", p=128))
                        sq = strp.tile([128, 2, TCH], f32, tag="sq", bufs=2)
                        nc.scalar.square(sq, xtile)
                        if kp == 0:
                            nc.vector.tensor_scalar_mul(sq_acc, sq, 1.0)
                        else:
                            nc.vector.tensor_tensor(sq_acc, sq_acc, sq,
                                                    ALU.add)
                        for g in range(2):
                            k = 2 * kp + g
                            for f in range(NF):
                                nc.tensor.matmul(
                                    ps_qkv[f],
                                    w_sb[:, k, f * 128:(f + 1) * 128],
                                    xtile[:, g, :], start=(k == 0),
                                    stop=(k == KT - 1),
                                    skip_group_check=True)
                    ssf = smallp.tile([128, TCH], f32, tag="ssf", bufs=1)
                    nc.vector.tensor_tensor(ssf, sq_acc[:, 0, :],
                                            sq_acc[:, 1, :], ALU.add)
                    ssr = smallp.tile([128, TCH], f32, tag="ssr", bufs=1)
                    nc.gpsimd.partition_all_reduce(ssr, ssf, 128,
                                                   bass_isa.ReduceOp.add)
                    ms = smallp.tile([128, TCH], f32, tag="ms", bufs=1)
                    nc.scalar.activation(ms, ssr, AF.Sqrt,
                                         bias=eps_col, scale=1.0 / H)
                    rb = smallp.tile([128, TCH], f32, tag="rb_a")
                    nc.vector.reciprocal(rb, ms)
                    for f in range(FQK):
                        nc.vector.tensor_tensor(
                            qk_sb[:, f, t0:t0 + TCH].bitcast(f32r),
                            ps_qkv[f], rb, ALU.mult)
                    v_t = tmpp.tile([128, TCH], f32, tag="v_t", bufs=2)
                    nc.vector.tensor_tensor(v_t, ps_qkv[NF - 1], rb, ALU.mult)
                    for j in range(TCH // 128):
                        ps_tr = psp.tile([128, 128], f32, tag="tr", bufs=1)
                        nc.tensor.transpose(
                            ps_tr, v_t[:, j * 128:(j + 1) * 128], ident)
                        nc.scalar.copy(
                            v_tok[:, (t0 // 128) + j, :].bitcast(f32r), ps_tr)

                    # ---- RoPE on this chunk (overlaps next chunk's PE) ----
                    hd = D // 2
                    for f in range(FQK):
                        qs = qk_sb[:, f, t0:t0 + TCH]
                        tmp = tmpp.tile([128, TCH], f32, tag="rope_tmp",
                                        bufs=2)
                        nc.scalar.copy(tmp[0:hd, :],
                                       qk_sb[hd:D, f, t0:t0 + TCH])
                        nc.scalar.copy(tmp[hd:D, :],
                                       qk_sb[0:hd, f, t0:t0 + TCH])
                        nc.vector.tensor_tensor(tmp, tmp,
                                                sn_sb[:, t0:t0 + TCH],
                                                ALU.mult)
                        nc.vector.tensor_tensor(qs.bitcast(f32r), qs,
                                                cs_sb[:, t0:t0 + TCH],
                                                ALU.mult)
                        nc.vector.tensor_tensor(qs.bitcast(f32r), qs, tmp,
                                                ALU.add)

                # ---- phase B: attention (two heads in flight so one
                # stream's exp hides under the other's matmuls) ----
                for b in range(B):
                    for qc in range(NQC):
                        qt0 = qc * TQ
                        nkt = (qt0 + TQ) // 128
                        diag0 = nkt - DIAG
                        for hp in range(NQH // 2):
                            hs = (2 * hp, 2 * hp + 1)
                            ps_o = [psp.tile([128, TQ], f32, tag="acc",
                                             bufs=6, name=f"ps_o{i}")
                                    for i in range(2)]
                            dens = [psp.tile([1, TQ], f32, tag="ss", bufs=1,
                                             name="den0"),
                                    psp.tile([1, TQ], f32, tag="tr", bufs=1,
                                             name="den1")]
                            for kt in range(nkt):
                                es = []
                                for i, h in enumerate(hs):
                                    ps_s = psp.tile([128, TQ], f32,
                                                    tag="acc", bufs=6)
                                    mmr(ps_s,
                                        qk_sb[:, NQH, b * S + kt * 128:
                                              b * S + (kt + 1) * 128],
                                        qk_sb[:, h,
                                              b * S + qt0:b * S + qt0 + TQ],
                                        start=True, stop=True,
                                        skip_group_check=True)
                                    e_sb = tmpp.tile([128, TQ], f32,
                                                     tag="e_sb", bufs=4)
                                    nc.scalar.activation(
                                        e_sb.bitcast(f32r), ps_s, AF.Exp,
                                        scale=sm_scale)
                                    if kt >= diag0:
                                        nc.vector.tensor_tensor(
                                            e_sb.bitcast(f32r), e_sb,
                                            msk_sb[:, kt - diag0, :],
                                            ALU.mult)
                                    es.append(e_sb)
                                for i in range(2):
                                    mmr(ps_o[i], v_tok[:, b * SB + kt, :],
                                        es[i], start=(kt == 0),
                                        stop=(kt == nkt - 1),
                                        skip_group_check=True)
                                    mmr(dens[i], ones_col, es[i],
                                        start=(kt == 0),
                                        stop=(kt == nkt - 1),
                                        skip_group_check=True)
                            for i, h in enumerate(hs):
                                rden = smallp.tile([1, TQ], f32, tag="rden")
                                nc.vector.reciprocal(rden, dens[i])
                                rbd = smallp.tile([128, TQ], f32, tag="rbd")
                                nc.gpsimd.partition_broadcast(rbd, rden, 128)
                                nc.vector.tensor_tensor(
                                    attn_sb[:, h,
                                            b * S + qt0:b * S + qt0 + TQ],
                                    ps_o[i], rbd, ALU.mult)

                # ---- phase C1: o-proj + 4-way split RS1 (early quarters
                # reduce while later quarters still compute) ----
                qM1 = MH // 4
                for m in range(MH):
                    wos = strp.tile([128, NQH * 128], bf16, tag="wo_slab",
                                    bufs=6)
                    nc.sync.dma_start(wos, wo.ap()[m])
                    tgt = bounce1q[m // qM1]
                    mrow = (m % qM1) * 128
                    for tci in range(NTC):
                        ps = psp.tile([128, TCH], f32, tag="acc", bufs=6)
                        for kh in range(NQH):
                            nc.tensor.matmul(
                                ps, wos[:, kh * 128:(kh + 1) * 128],
                                attn_sb[:, kh, tci * TCH:(tci + 1) * TCH],
                                start=(kh == 0), stop=(kh == NQH - 1))
                        ob = tmpp.tile([128, TCH], bf16, tag="o_bf", bufs=4)
                        nc.vector.tensor_scalar_mul(ob, ps, 1.0)
                        nc.scalar.dma_start(
                            tgt[tci * NSH:(tci + 1) * NSH,
                                mrow:mrow + 128, :].rearrange(
                                    "s p t -> p s t"),
                            ob.rearrange("p (s t) -> p s t", s=NSH))
                    if (m + 1) % qM1 == 0:
                        q = m // qM1
                        nc.gpsimd.collective_compute(
                            "ReduceScatter", ALU.add, replica_groups=rg,
                            ins=[bounce1q[q].opt()], outs=[rs1q[q].opt()])

            # ============ scope 2: norm2 + AG + MLP + final ============
            with (
                tc.tile_pool(name="bigCD", bufs=1) as bigp2,
                tc.tile_pool(name="strCD", bufs=4) as strp2,
                tc.tile_pool(name="tmpCD", bufs=2) as tmpp2,
            ):
                h_sh = bigp2.tile([128, KT, TS], f32, tag="h_sh")
                act_sb = bigp2.tile([128, KI, T], bf16, tag="act")

                qK1 = KT // 4
                ps_ss2 = psp.tile([1, TS], f32, bufs=1, tag="ss")
                for kp in range(KT // 2):
                    k0 = 2 * kp
                    srcb = rs1q[k0 // qK1]
                    krow = (k0 % qK1) * 128
                    rt = strp2.tile([128, 2, TS], bf16, tag="rs1_t", bufs=2)
                    nc.sync.dma_start(
                        rt, srcb[krow:krow + 256, :].rearrange(
                            "(g p) t -> p g t", p=128))
                    xst = strp2.tile([128, 2, TS], f32, tag="xs_t", bufs=2)
                    nc.sync.dma_start(
                        xst, xs.ap()[k0 * 128:(k0 + 2) * 128, :].rearrange(
                            "(g p) t -> p g t", p=128))
                    nc.vector.tensor_tensor(
                        h_sh[:, k0:k0 + 2, :], rt, xst, ALU.add)
                    sq2 = strp2.tile([128, 2, TS], f32, tag="sq2", bufs=2)
                    nc.scalar.square(sq2.bitcast(f32r), h_sh[:, k0:k0 + 2, :])
                    for g in range(2):
                        k = k0 + g
                        mmr(ps_ss2, ones_col, sq2[:, g, :],
                            start=(k == 0), stop=(k == KT - 1),
                            skip_group_check=True)
                ms2 = smallp.tile([1, TS], f32, tag="ms2")
                nc.scalar.activation(ms2, ps_ss2, AF.Sqrt,
                                     bias=eps_col[0:1, :], scale=1.0 / H)
                rr2 = smallp.tile([1, TS], f32, tag="rr2")
                nc.vector.reciprocal(rr2, ms2)
                rb2 = smallp.tile([128, TS], f32, tag="rb2")
                nc.gpsimd.partition_broadcast(rb2, rr2, 128)
                for k in range(KT):
                    mib = tmpp2.tile([128, TS], bf16, tag="mib", bufs=4)
                    nc.vector.tensor_tensor(mib, h_sh[:, k, :], rb2, ALU.mult)
                    nc.scalar.dma_start(ag_in[k * 128:(k + 1) * 128, :], mib)
                nc.gpsimd.collective_compute(
                    "AllGather", ALU.bypass, replica_groups=rg,
                    ins=[ag_in.opt()], outs=[ag_out.opt()])

                # ---- phase D: gate/up ----
                for tci in range(NTC):
                    t0 = tci * TCH
                    # two half-k tiles on a 3-deep rotation: the next
                    # chunk's first half prefetches while this chunk's
                    # matmuls still read both halves
                    mi_h = [strp2.tile([128, KT // 2, TCH], bf16, tag="mi",
                                       bufs=3, name=f"mi{h}")
                            for h in range(2)]
                    for k in range(KT):
                        src = ag_out[tci * NSH:(tci + 1) * NSH,
                                     k * 128:(k + 1) * 128, :]
                        nc.sync.dma_start(
                            mi_h[k // (KT // 2)][:, k % (KT // 2), :]
                            .rearrange("p (b t) -> p b t", b=NSH),
                            src.rearrange("b p t -> p b t"))
                    for fi in range(KI):
                        wg = strp2.tile([128, KT * 128], bf16, tag="wg_slab",
                                        bufs=2)
                        nc.sync.dma_start(wg, wgu.ap()[fi])
                        wu = strp2.tile([128, KT * 128], bf16, tag="wu_slab",
                                        bufs=1)
                        nc.sync.dma_start(wu, wgu.ap()[KI + fi])
                        ps_g = psp.tile([128, TCH], f32, tag="acc", bufs=6)
                        ps_u = psp.tile([128, TCH], f32, tag="acc", bufs=6)
                        for k in range(KT):
                            nc.tensor.matmul(
                                ps_g, wg[:, k * 128:(k + 1) * 128],
                                mi_h[k // (KT // 2)][:, k % (KT // 2), :],
                                start=(k == 0), stop=(k == KT - 1),
                                skip_group_check=True)
                        for k in range(KT):
                            nc.tensor.matmul(
                                ps_u, wu[:, k * 128:(k + 1) * 128],
                                mi_h[k // (KT // 2)][:, k % (KT // 2), :],
                                start=(k == 0), stop=(k == KT - 1),
                                skip_group_check=True)
                        sg = tmpp2.tile([128, TCH], f32, tag="sg", bufs=2)
                        nc.scalar.activation(sg, ps_g, AF.Silu)
                        nc.vector.tensor_tensor(act_sb[:, fi, t0:t0 + TCH],
                                                sg, ps_u, ALU.mult)

                # ---- phase D2: down-proj + 4-way split RS2 + residual ----
                qM = MH // 4
                for m in range(MH):
                    wds = strp2.tile([128, KI * 128], bf16, tag="wd_slab",
                                     bufs=3)
                    nc.sync.dma_start(wds, wdn.ap()[m])
                    tgt = bounce2q[m // qM]
                    mrow = (m % qM) * 128
                    for tci in range(NTC):
                        ps = psp.tile([128, TCH], f32, tag="acc", bufs=6)
                        for k in range(KI):
                            nc.tensor.matmul(
                                ps, wds[:, k * 128:(k + 1) * 128],
                                act_sb[:, k, tci * TCH:(tci + 1) * TCH],
                                start=(k == 0), stop=(k == KI - 1))
                        db = tmpp2.tile([128, TCH], bf16, tag="d_bf", bufs=4)
                        nc.vector.tensor_scalar_mul(db, ps, 1.0)
                        nc.scalar.dma_start(
                            tgt[tci * NSH:(tci + 1) * NSH,
                                mrow:mrow + 128, :].rearrange(
                                    "s p t -> p s t"),
                            db.rearrange("p (s t) -> p s t", s=NSH))
                    if (m + 1) % qM == 0:
                        q = m // qM
                        nc.gpsimd.collective_compute(
                            "ReduceScatter", ALU.add, replica_groups=rg,
                            ins=[bounce2q[q].opt()], outs=[rs2q[q].opt()])

                qK = KT // 4
                for k in range(KT):
                    r2t = strp2.tile([128, TS], bf16, tag="rs2_t", bufs=4)
                    nc.sync.dma_start(
                        r2t, rs2q[k // qK][(k % qK) * 128:(k % qK) * 128 + 128, :])
                    ot = tmpp2.tile([128, TS], f32, tag="out_t", bufs=4)
                    nc.vector.tensor_tensor(ot, r2t, h_sh[:, k, :], ALU.add)
                    nc.scalar.dma_start(out_t.ap()[k * 128:(k + 1) * 128, :],
                                        ot)

    nc.compile()
    return nc


def prepare_inputs(inputs, cfg):
    """Full np inputs -> per-core in_maps (host-side sharding/prep)."""
    H, S, B = cfg["H"], cfg["S"], cfg["B"]
    NQH, D, IC = cfg["NQH"], cfg["D"], cfg["IC"]
    NCORES = cfg["NC"]
    T = B * S
    TS = T // NCORES
    TQ = min(512, S)
    DIAG = TQ // 128
    f4 = np.float32

    x = np.asarray(inputs["x"], f4).reshape(T, H)
    xt = np.ascontiguousarray(x.T)                      # [H, T]
    cos = np.asarray(inputs["cos"], f4)                 # [S, D]
    sin = np.asarray(inputs["sin"], f4)
    cs = np.ascontiguousarray(np.tile(cos.T, (1, B)))   # [D, T]
    sn_s = sin.T.copy()
    sn_s[: D // 2] *= -1.0                              # sign-folded rot_half
    sn = np.ascontiguousarray(np.tile(sn_s, (1, B)))
    anw = np.asarray(inputs["attn_norm_w"], f4)
    fnw = np.asarray(inputs["ffn_norm_w"], f4)
    wq = np.asarray(inputs["wq"], f4) * anw[:, None]
    wk = np.asarray(inputs["wk"], f4) * anw[:, None]
    wv = np.asarray(inputs["wv"], f4) * anw[:, None]
    # wqkv ships bf16 (kernel keeps it SBUF-resident)
    wo = np.asarray(inputs["wo"], f4)
    wg = np.asarray(inputs["w_gate"], f4) * fnw[:, None]
    wu = np.asarray(inputs["w_up"], f4) * fnw[:, None]
    wd = np.asarray(inputs["w_down"], f4)
    I_full = wg.shape[1]
    I_pad = NCORES * IC
    if I_pad > I_full:
        pad = I_pad - I_full
        wg = np.pad(wg, ((0, 0), (0, pad)))
        wu = np.pad(wu, ((0, 0), (0, pad)))
        wd = np.pad(wd, ((0, pad), (0, 0)))

    # causal masks, transposed layout: keep (j*128 + kt) <= q
    kt_i = np.arange(128)[:, None]
    q_i = np.arange(TQ)[None, :]
    msk = np.stack([(j * 128 + kt_i <= q_i).astype(f4) for j in range(DIAG)])

    def pack_slabs(w):
        # [K*128, ncols] -> [ncols//128, 128, K*128]: one contiguous
        # DRAM block per 128-column output slab
        K = w.shape[0] // 128
        nc_ = w.shape[1] // 128
        return np.ascontiguousarray(
            w.reshape(K, 128, nc_, 128).transpose(2, 1, 0, 3)
            .reshape(nc_, 128, K * 128))

    qd, kvd = NQH * D, D  # per-core q cols, kv cols
    in_maps = []
    for c in range(NCORES):
        wqkv = np.concatenate([
            wq[:, c * qd:(c + 1) * qd],
            wk[:, c * kvd:(c + 1) * kvd],
            wv[:, c * kvd:(c + 1) * kvd]], axis=1)
        wgu = np.concatenate([
            pack_slabs(wg[:, c * IC:(c + 1) * IC]),
            pack_slabs(wu[:, c * IC:(c + 1) * IC])], axis=0).astype(BF16)
        in_maps.append({
            "xt": xt,
            "xs": np.ascontiguousarray(xt[:, c * TS:(c + 1) * TS]),
            "wqkv": np.ascontiguousarray(wqkv.astype(BF16)),
            "wo": pack_slabs(wo[c * qd:(c + 1) * qd, :]).astype(BF16),
            "wgu": np.ascontiguousarray(wgu),
            "wdn": pack_slabs(wd[c * IC:(c + 1) * IC, :]).astype(BF16),
            "cs": cs, "sn": sn, "msk": msk,
        })
    return in_maps


def assemble_output(results, cfg):
    H, S, B, NCORES = cfg["H"], cfg["S"], cfg["B"], cfg["NC"]
    full_t = np.concatenate([r["out_t"] for r in results], axis=1)  # [H, T]
    return np.ascontiguousarray(full_t.T).reshape(B, S, H)


_NC_CACHE = {}


def _get_nc(cfg_key, cfg):
    if cfg_key not in _NC_CACHE:
        _NC_CACHE[cfg_key] = build_nc(cfg)
    return _NC_CACHE[cfg_key]


def run(inputs, cfg, **kwargs):
    nc = _get_nc(tuple(sorted(cfg.items())), cfg)
    in_maps = prepare_inputs(inputs, cfg)
    res = run_bass_kernel_spmd(nc, in_maps,
                               core_ids=list(range(cfg["NC"])), **kwargs)
    return assemble_output(res.results, cfg), res


def kernel(**inputs) -> np.ndarray:
    out, _ = run(inputs, FULL_CFG)
    return out



# revision 52
# speedup vs baseline: 2.3899x; 2.0443x over previous
"""Tensor-parallel LlamaDecoderLayer forward on 8 Trainium2 NeuronCores.

Sharding (per the TP hint):
- attention: 4 q-heads + 1 kv-head per core (GQA groups align with cores);
  o-proj row-sharded; partial outputs reduce-scattered over tokens (bf16 wire)
- norm2 computed on each core's 256-token shard; normalized activations
  all-gathered (bf16)
- MLP: gate/up column-sharded / down row-sharded over the intermediate dim
  (zero-padded 11008 -> 11264 so every core gets 1408 = 11*128);
  down partials reduce-scattered over tokens; final residual added on the
  token shard and returned per-core, assembled on host.

Layout: activations are kept feature-major ([feature, token], feature on
SBUF partitions) so every weight matrix loads as lhsT in its natural layout.
RMSNorm is folded into the matmul epilogue (scale columns of the product by
the per-token rms), softmax runs in transposed [k_tok, q_tok] layout without
max-subtraction (scores are bounded; fp32 exp cannot overflow), and
denominators come from ones-vector matmuls (partition reduction on the PE).

Performance structure:
- all matmuls run at 1 PE cycle/row: bf16 for qkv/o-proj/MLP, fp32r (raw
  fp32 streaming) for attention scores/AV and norm2's sum-of-squares;
  phase A's rmsnorm reduction runs off the PE entirely (DVE accumulates
  squares across k-tiles, one gpsimd partition_all_reduce per chunk whose
  all-partition output also replaces the reciprocal broadcast)
- wqkv is bf16 and SBUF-resident (loaded once); x streams via gpsimd
  casting DMAs (f32 DRAM -> bf16 SBUF); wo/wgu/wdn are packed host-side
  into contiguous per-slab blocks so weight DMA bursts are >= 1KB
- RoPE is applied per token chunk inside phase A (overlaps next chunk's PE)
- attention runs two heads in flight so one stream's exp hides under the
  other's matmuls; per-stream denominators use separate PSUM banks
- RS1 and RS2 split in feature quarters: early quarters reduce while
  later o-proj/down-proj quarters still compute, and only a quarter of
  the dependent work waits for the last piece; the MLP AllGather stays
  single (splitting it can't overlap enough compute to amortize the
  extra launch)
- PSUM->bf16 spills run on DVE; write-back DMAs issue from the Activation
  HWDGE queue so weight prefetches on the SP queue never starve the PE
"""

import numpy as np
import ml_dtypes

import concourse.bacc as bacc
import concourse.bass as bass
import concourse.bass_isa as bass_isa
import concourse.mybir as mybir
import concourse.tile as tile
from concourse.bass_utils import run_bass_kernel_spmd
from concourse.masks import make_identity

AF = mybir.ActivationFunctionType
ALU = mybir.AluOpType
DT = mybir.dt
BF16 = ml_dtypes.bfloat16

FULL_CFG = dict(H=4096, S=1024, B=2, NQH=4, D=128, IC=1408, NC=8, EPS=1e-5)


def build_nc(cfg):
    H, S, B = cfg["H"], cfg["S"], cfg["B"]
    NQH, D, IC = cfg["NQH"], cfg["D"], cfg["IC"]
    NCORES, EPS = cfg["NC"], cfg["EPS"]
    T = B * S
    KT = H // 128          # hidden-dim k tiles
    FQK = NQH + 1          # q tiles + 1 k tile (feature-major outputs)
    NF = FQK + 1           # + v tile -> qkv feature tiles
    KI = IC // 128         # intermediate k tiles (per-core shard)
    TS = T // NCORES       # token shard (reduce-scatter granularity)
    TQ = min(512, S)       # attention query chunk
    NQC = S // TQ
    DIAG = TQ // 128       # diagonal (masked) kt blocks per query chunk
    TCH = min(512, T)      # matmul token chunk
    NTC = T // TCH
    NSH = TCH // TS if TCH >= TS else 1   # shard blocks per token chunk
    MH = H // 128          # output feature tiles
    SB = S // 128          # seq kt blocks per batch
    sm_scale = float(1.0 / np.sqrt(D))
    f32, bf16 = DT.float32, DT.bfloat16

    nc = bacc.Bacc("TRN2", target_bir_lowering=False, debug=False,
                   num_devices=NCORES)

    f32r = DT.float32r

    def mmr(out, lhsT, rhs, **kw):
        # fp32 operands streamed in raw mode: 1 PE cycle/row at free dim
        # >= 256 (vs 4 for decomposed fp32), bf16-class operand precision
        nc.tensor.matmul(out, lhsT.bitcast(f32r), rhs.bitcast(f32r), **kw)

    xt = nc.dram_tensor("xt", [H, T], f32, kind="ExternalInput")
    xs = nc.dram_tensor("xs", [H, TS], f32, kind="ExternalInput")
    wqkv = nc.dram_tensor("wqkv", [H, NF * 128], bf16, kind="ExternalInput")
    # weight slabs packed host-side: each [128, K*128] slab is one
    # contiguous DRAM block (256B-segment reads halve DMA bus efficiency)
    wo = nc.dram_tensor("wo", [MH, 128, NQH * 128], bf16,
                        kind="ExternalInput")
    wgu = nc.dram_tensor("wgu", [2 * KI, 128, KT * 128], bf16,
                         kind="ExternalInput")
    wdn = nc.dram_tensor("wdn", [MH, 128, KI * 128], bf16,
                         kind="ExternalInput")
    cs = nc.dram_tensor("cs", [D, T], f32, kind="ExternalInput")
    sn = nc.dram_tensor("sn", [D, T], f32, kind="ExternalInput")
    msk = nc.dram_tensor("msk", [DIAG, 128, TQ], f32, kind="ExternalInput")
    out_t = nc.dram_tensor("out_t", [H, TS], f32, kind="ExternalOutput")

    wqkv_r = wqkv.ap().rearrange("(ko p) f -> p ko f", p=128)
    rg = [list(range(NCORES))]

    with tile.TileContext(nc, num_cores=NCORES) as tc:
        with (
            tc.tile_pool(name="misc", bufs=1) as miscp,
            tc.tile_pool(name="small", bufs=2) as smallp,
            tc.tile_pool(name="dram", bufs=1, space="DRAM") as dramp,
            tc.tile_pool(name="ps", bufs=1, space="PSUM") as psp,
        ):
            ones_f = miscp.tile([128, 1], f32, tag="ones_f")
            nc.gpsimd.memset(ones_f, 1.0)
            ones_col = miscp.tile([128, 1], f32, tag="ones_col")
            nc.scalar.copy(ones_col.bitcast(f32r), ones_f)
            eps_col = miscp.tile([128, 1], f32, tag="eps_col")
            nc.gpsimd.memset(eps_col, EPS)

            # reduce-scatters split along features so early parts overlap
            # with compute; the AllGather stays whole (nothing to overlap)
            HQ = H // 4
            bounce1q = [dramp.tile([NCORES, HQ, TS], bf16,
                                   tag=f"bounce1q{q}", name=f"bounce1q{q}")
                        for q in range(4)]
            rs1q = [dramp.tile([HQ, TS], bf16, tag=f"rs1q{q}",
                               name=f"rs1q{q}")
                    for q in range(4)]
            ag_in = dramp.tile([H, TS], bf16, tag="ag_in")
            ag_out = dramp.tile([NCORES, H, TS], bf16, tag="ag_out",
                                addr_space="Shared")
            bounce2q = [dramp.tile([NCORES, HQ, TS], bf16,
                                   tag=f"bounce2q{q}", name=f"bounce2q{q}")
                        for q in range(4)]
            rs2q = [dramp.tile([HQ, TS], bf16, tag=f"rs2q{q}",
                               name=f"rs2q{q}")
                    for q in range(4)]

            # ============ scope 1: qkv + attention + o-proj ============
            with (
                tc.tile_pool(name="bigAB", bufs=1) as bigp,
                tc.tile_pool(name="strAB", bufs=4) as strp,
                tc.tile_pool(name="tmpAB", bufs=3) as tmpp,
            ):
                ident = bigp.tile([128, 128], f32, tag="ident")
                make_identity(nc, ident)
                cs_sb = bigp.tile([128, T], f32, tag="cs")
                sn_sb = bigp.tile([128, T], f32, tag="sn")
                for i in range(T // 512):
                    nc.sync.dma_start(cs_sb[:, i * 512:(i + 1) * 512],
                                      cs.ap()[:, i * 512:(i + 1) * 512])
                    nc.sync.dma_start(sn_sb[:, i * 512:(i + 1) * 512],
                                      sn.ap()[:, i * 512:(i + 1) * 512])
                msk_sb = bigp.tile([128, DIAG, TQ], f32, tag="msk")
                for j in range(DIAG):
                    nc.sync.dma_start(msk_sb[:, j, :], msk.ap()[j])

                qk_sb = bigp.tile([128, FQK, T], f32, tag="qk")
                v_tok = bigp.tile([128, T // 128, 128], f32, tag="vtok")
                attn_sb = bigp.tile([128, NQH, T], bf16, tag="attn")

                # wqkv (bf16) resident in SBUF: loaded once, reused by all
                # token chunks (re-streaming fp32 weights made phase A
                # DMA-bound)
                w_sb = bigp.tile([128, KT, NF * 128], bf16, tag="w_sb")
                for k in range(KT):
                    nc.sync.dma_start(w_sb[:, k, :], wqkv_r[:, k, :])

                # ---- phase A: rmsnorm-folded qkv ----
                for tci in range(NTC):
                    t0 = tci * TCH
                    ps_qkv = [psp.tile([128, TCH], f32, tag="acc", bufs=6,
                                       name=f"ps_qkv{f}")
                              for f in range(NF)]
                    sq_acc = strp.tile([128, 2, TCH], f32, tag="sq_acc",
                                       bufs=1)
                    for kp in range(KT // 2):
                        xtile = strp.tile([128, 2, TCH], bf16, tag="x_in",
                                          bufs=3)
                        nc.gpsimd.dma_start(
                            xtile,
                            xt.ap()[kp * 256:(kp + 1) * 256,
                                    t0:t0 + TCH].rearrange(
                                        "(g p) t -> p g t", p=128))
                        sq = strp.tile([128, 2, TCH], f32, tag="sq", bufs=2)
                        nc.scalar.square(sq, xtile)
                        if kp == 0:
                            nc.vector.tensor_scalar_mul(sq_acc, sq, 1.0)
                        else:
                            nc.vector.tensor_tensor(sq_acc, sq_acc, sq,
                                                    ALU.add)
                        for g in range(2):
                            k = 2 * kp + g
                            for f in range(NF):
                                nc.tensor.matmul(
                                    ps_qkv[f],
                                    w_sb[:, k, f * 128:(f + 1) * 128],
                                    xtile[:, g, :], start=(k == 0),
                                    stop=(k == KT - 1),
                                    skip_group_check=True)
                    ssf = smallp.tile([128, TCH], f32, tag="ssf", bufs=1)
                    nc.vector.tensor_tensor(ssf, sq_acc[:, 0, :],
                                            sq_acc[:, 1, :], ALU.add)
                    ssr = smallp.tile([128, TCH], f32, tag="ssr", bufs=1)
                    nc.gpsimd.partition_all_reduce(ssr, ssf, 128,
                                                   bass_isa.ReduceOp.add)
                    ms = smallp.tile([128, TCH], f32, tag="ms", bufs=1)
                    nc.scalar.activation(ms, ssr, AF.Sqrt,
                                         bias=eps_col, scale=1.0 / H)
                    rb = smallp.tile([128, TCH], f32, tag="rb_a")
                    nc.vector.reciprocal(rb, ms)
                    for f in range(FQK):
                        nc.vector.tensor_tensor(
                            qk_sb[:, f, t0:t0 + TCH].bitcast(f32r),
                            ps_qkv[f], rb, ALU.mult)
                    v_t = tmpp.tile([128, TCH], f32, tag="v_t", bufs=2)
                    nc.vector.tensor_tensor(v_t, ps_qkv[NF - 1], rb, ALU.mult)
                    for j in range(TCH // 128):
                        ps_tr = psp.tile([128, 128], f32, tag="tr", bufs=1)
                        nc.tensor.transpose(
                            ps_tr, v_t[:, j * 128:(j + 1) * 128], ident)
                        nc.scalar.copy(
                            v_tok[:, (t0 // 128) + j, :].bitcast(f32r), ps_tr)

                    # ---- RoPE on this chunk (overlaps next chunk's PE) ----
                    hd = D // 2
                    for f in range(FQK):
                        qs = qk_sb[:, f, t0:t0 + TCH]
                        tmp = tmpp.tile([128, TCH], f32, tag="rope_tmp",
                                        bufs=2)
                        nc.scalar.copy(tmp[0:hd, :],
                                       qk_sb[hd:D, f, t0:t0 + TCH])
                        nc.scalar.copy(tmp[hd:D, :],
                                       qk_sb[0:hd, f, t0:t0 + TCH])
                        nc.vector.tensor_tensor(tmp, tmp,
                                                sn_sb[:, t0:t0 + TCH],
                                                ALU.mult)
                        nc.vector.tensor_tensor(qs.bitcast(f32r), qs,
                                                cs_sb[:, t0:t0 + TCH],
                                                ALU.mult)
                        nc.vector.tensor_tensor(qs.bitcast(f32r), qs, tmp,
                                                ALU.add)

                # ---- phase B: attention (two heads in flight so one
                # stream's exp hides under the other's matmuls; each
                # group's softmax epilogue is deferred into the next
                # group's loop so its chain overlaps compute) ----
                pend_ep = []   # (ps_o pair, dens pair, hs, b, qt0)

                def emit_attn_epilogue():
                    for po, dn, phs, pb, pq in pend_ep:
                        for i, h in enumerate(phs):
                            rden = smallp.tile([1, TQ], f32, tag="rden")
                            nc.vector.reciprocal(rden, dn[i])
                            rbd = smallp.tile([128, TQ], f32, tag="rbd")
                            nc.gpsimd.partition_broadcast(rbd, rden, 128)
                            nc.vector.tensor_tensor(
                                attn_sb[:, h, pb * S + pq:pb * S + pq + TQ],
                                po[i], rbd, ALU.mult)
                    pend_ep.clear()

                for b in range(B):
                    for qc in range(NQC):
                        qt0 = qc * TQ
                        nkt = (qt0 + TQ) // 128
                        diag0 = nkt - DIAG
                        for hp in range(NQH // 2):
                            hs = (2 * hp, 2 * hp + 1)
                            ps_o = [psp.tile([128, TQ], f32, tag="acc",
                                             bufs=6, name=f"ps_o{i}")
                                    for i in range(2)]
                            dens = [psp.tile([1, TQ], f32, tag="ss", bufs=1,
                                             name="den0"),
                                    psp.tile([1, TQ], f32, tag="tr", bufs=1,
                                             name="den1")]
                            for kt in range(nkt):
                                es = []
                                for i, h in enumerate(hs):
                                    ps_s = psp.tile([128, TQ], f32,
                                                    tag="acc", bufs=6)
                                    mmr(ps_s,
                                        qk_sb[:, NQH, b * S + kt * 128:
                                              b * S + (kt + 1) * 128],
                                        qk_sb[:, h,
                                              b * S + qt0:b * S + qt0 + TQ],
                                        start=True, stop=True,
                                        skip_group_check=True)
                                    e_sb = tmpp.tile([128, TQ], f32,
                                                     tag="e_sb", bufs=4)
                                    nc.scalar.activation(
                                        e_sb.bitcast(f32r), ps_s, AF.Exp,
                                        scale=sm_scale)
                                    if kt >= diag0:
                                        nc.vector.tensor_tensor(
                                            e_sb.bitcast(f32r), e_sb,
                                            msk_sb[:, kt - diag0, :],
                                            ALU.mult)
                                    es.append(e_sb)
                                if kt == 0:
                                    # previous group's epilogue: its
                                    # reciprocals free the den banks
                                    # before this group's den matmuls
                                    emit_attn_epilogue()
                                for i in range(2):
                                    mmr(ps_o[i], v_tok[:, b * SB + kt, :],
                                        es[i], start=(kt == 0),
                                        stop=(kt == nkt - 1),
                                        skip_group_check=True)
                                    mmr(dens[i], ones_col, es[i],
                                        start=(kt == 0),
                                        stop=(kt == nkt - 1),
                                        skip_group_check=True)
                            pend_ep.append((ps_o, dens, hs, b, qt0))
                emit_attn_epilogue()

                # ---- phase C1: o-proj + 4-way split RS1 (early quarters
                # reduce while later quarters still compute) ----
                qM1 = MH // 4
                for m in range(MH):
                    wos = strp.tile([128, NQH * 128], bf16, tag="wo_slab",
                                    bufs=6)
                    nc.sync.dma_start(wos, wo.ap()[m])
                    tgt = bounce1q[m // qM1]
                    mrow = (m % qM1) * 128
                    for tci in range(NTC):
                        ps = psp.tile([128, TCH], f32, tag="acc", bufs=6)
                        for kh in range(NQH):
                            nc.tensor.matmul(
                                ps, wos[:, kh * 128:(kh + 1) * 128],
                                attn_sb[:, kh, tci * TCH:(tci + 1) * TCH],
                                start=(kh == 0), stop=(kh == NQH - 1))
                        ob = tmpp.tile([128, TCH], bf16, tag="o_bf", bufs=4)
                        nc.vector.tensor_scalar_mul(ob, ps, 1.0)
                        nc.scalar.dma_start(
                            tgt[tci * NSH:(tci + 1) * NSH,
                                mrow:mrow + 128, :].rearrange(
                                    "s p t -> p s t"),
                            ob.rearrange("p (s t) -> p s t", s=NSH))
                    if (m + 1) % qM1 == 0:
                        q = m // qM1
                        nc.gpsimd.collective_compute(
                            "ReduceScatter", ALU.add, replica_groups=rg,
                            ins=[bounce1q[q].opt()], outs=[rs1q[q].opt()])

            # ============ scope 2: norm2 + AG + MLP + final ============
            with (
                tc.tile_pool(name="bigCD", bufs=1) as bigp2,
                tc.tile_pool(name="strCD", bufs=4) as strp2,
                tc.tile_pool(name="tmpCD", bufs=2) as tmpp2,
            ):
                h_sh = bigp2.tile([128, KT, TS], f32, tag="h_sh")
                act_sb = bigp2.tile([128, KI, T], bf16, tag="act")

                qK1 = KT // 4
                ps_ss2 = psp.tile([1, TS], f32, bufs=1, tag="ss")
                for kp in range(KT // 2):
                    k0 = 2 * kp
                    srcb = rs1q[k0 // qK1]
                    krow = (k0 % qK1) * 128
                    rt = strp2.tile([128, 2, TS], bf16, tag="rs1_t", bufs=2)
                    nc.sync.dma_start(
                        rt, srcb[krow:krow + 256, :].rearrange(
                            "(g p) t -> p g t", p=128))
                    xst = strp2.tile([128, 2, TS], f32, tag="xs_t", bufs=2)
                    nc.sync.dma_start(
                        xst, xs.ap()[k0 * 128:(k0 + 2) * 128, :].rearrange(
                            "(g p) t -> p g t", p=128))
                    nc.vector.tensor_tensor(
                        h_sh[:, k0:k0 + 2, :], rt, xst, ALU.add)
                    sq2 = strp2.tile([128, 2, TS], f32, tag="sq2", bufs=2)
                    nc.scalar.square(sq2.bitcast(f32r), h_sh[:, k0:k0 + 2, :])
                    for g in range(2):
                        k = k0 + g
                        mmr(ps_ss2, ones_col, sq2[:, g, :],
                            start=(k == 0), stop=(k == KT - 1),
                            skip_group_check=True)
                ms2 = smallp.tile([1, TS], f32, tag="ms2")
                nc.scalar.activation(ms2, ps_ss2, AF.Sqrt,
                                     bias=eps_col[0:1, :], scale=1.0 / H)
                rr2 = smallp.tile([1, TS], f32, tag="rr2")
                nc.vector.reciprocal(rr2, ms2)
                rb2 = smallp.tile([128, TS], f32, tag="rb2")
                nc.gpsimd.partition_broadcast(rb2, rr2, 128)
                for k in range(KT):
                    mib = tmpp2.tile([128, TS], bf16, tag="mib", bufs=4)
                    nc.vector.tensor_tensor(mib, h_sh[:, k, :], rb2, ALU.mult)
                    nc.scalar.dma_start(ag_in[k * 128:(k + 1) * 128, :], mib)
                nc.gpsimd.collective_compute(
                    "AllGather", ALU.bypass, replica_groups=rg,
                    ins=[ag_in.opt()], outs=[ag_out.opt()])

                # ---- phase D: gate/up ----
                for tci in range(NTC):
                    t0 = tci * TCH
                    # two half-k tiles on a 3-deep rotation: the next
                    # chunk's first half prefetches while this chunk's
                    # matmuls still read both halves
                    mi_h = [strp2.tile([128, KT // 2, TCH], bf16, tag="mi",
                                       bufs=3, name=f"mi{h}")
                            for h in range(2)]
                    for k in range(KT):
                        src = ag_out[tci * NSH:(tci + 1) * NSH,
                                     k * 128:(k + 1) * 128, :]
                        nc.sync.dma_start(
                            mi_h[k // (KT // 2)][:, k % (KT // 2), :]
                            .rearrange("p (b t) -> p b t", b=NSH),
                            src.rearrange("b p t -> p b t"))
                    for fi in range(KI):
                        wg = strp2.tile([128, KT * 128], bf16, tag="wg_slab",
                                        bufs=2)
                        nc.sync.dma_start(wg, wgu.ap()[fi])
                        wu = strp2.tile([128, KT * 128], bf16, tag="wu_slab",
                                        bufs=1)
                        nc.sync.dma_start(wu, wgu.ap()[KI + fi])
                        ps_g = psp.tile([128, TCH], f32, tag="acc", bufs=6)
                        ps_u = psp.tile([128, TCH], f32, tag="acc", bufs=6)
                        for k in range(KT):
                            nc.tensor.matmul(
                                ps_g, wg[:, k * 128:(k + 1) * 128],
                                mi_h[k // (KT // 2)][:, k % (KT // 2), :],
                                start=(k == 0), stop=(k == KT - 1),
                                skip_group_check=True)
                        for k in range(KT):
                            nc.tensor.matmul(
                                ps_u, wu[:, k * 128:(k + 1) * 128],
                                mi_h[k // (KT // 2)][:, k % (KT // 2), :],
                                start=(k == 0), stop=(k == KT - 1),
                                skip_group_check=True)
                        sg = tmpp2.tile([128, TCH], f32, tag="sg", bufs=2)
                        nc.scalar.activation(sg, ps_g, AF.Silu)
                        nc.vector.tensor_tensor(act_sb[:, fi, t0:t0 + TCH],
                                                sg, ps_u, ALU.mult)

                # ---- phase D2: down-proj + 4-way split RS2 + residual ----
                qM = MH // 4
                for m in range(MH):
                    wds = strp2.tile([128, KI * 128], bf16, tag="wd_slab",
                                     bufs=3)
                    nc.sync.dma_start(wds, wdn.ap()[m])
                    tgt = bounce2q[m // qM]
                    mrow = (m % qM) * 128
                    for tci in range(NTC):
                        ps = psp.tile([128, TCH], f32, tag="acc", bufs=6)
                        for k in range(KI):
                            nc.tensor.matmul(
                                ps, wds[:, k * 128:(k + 1) * 128],
                                act_sb[:, k, tci * TCH:(tci + 1) * TCH],
                                start=(k == 0), stop=(k == KI - 1))
                        db = tmpp2.tile([128, TCH], bf16, tag="d_bf", bufs=4)
                        nc.vector.tensor_scalar_mul(db, ps, 1.0)
                        nc.scalar.dma_start(
                            tgt[tci * NSH:(tci + 1) * NSH,
                                mrow:mrow + 128, :].rearrange(
                                    "s p t -> p s t"),
                            db.rearrange("p (s t) -> p s t", s=NSH))
                    if (m + 1) % qM == 0:
                        q = m // qM
                        nc.gpsimd.collective_compute(
                            "ReduceScatter", ALU.add, replica_groups=rg,
                            ins=[bounce2q[q].opt()], outs=[rs2q[q].opt()])

                qK = KT // 4
                for k in range(KT):
                    r2t = strp2.tile([128, TS], bf16, tag="rs2_t", bufs=4)
                    nc.sync.dma_start(
                        r2t, rs2q[k // qK][(k % qK) * 128:(k % qK) * 128 + 128, :])
                    ot = tmpp2.tile([128, TS], f32, tag="out_t", bufs=4)
                    nc.vector.tensor_tensor(ot, r2t, h_sh[:, k, :], ALU.add)
                    nc.scalar.dma_start(out_t.ap()[k * 128:(k + 1) * 128, :],
                                        ot)

    nc.compile()
    return nc


def prepare_inputs(inputs, cfg):
    """Full np inputs -> per-core in_maps (host-side sharding/prep)."""
    H, S, B = cfg["H"], cfg["S"], cfg["B"]
    NQH, D, IC = cfg["NQH"], cfg["D"], cfg["IC"]
    NCORES = cfg["NC"]
    T = B * S
    TS = T // NCORES
    TQ = min(512, S)
    DIAG = TQ // 128
    f4 = np.float32

    x = np.asarray(inputs["x"], f4).reshape(T, H)
    xt = np.ascontiguousarray(x.T)                      # [H, T]
    cos = np.asarray(inputs["cos"], f4)                 # [S, D]
    sin = np.asarray(inputs["sin"], f4)
    cs = np.ascontiguousarray(np.tile(cos.T, (1, B)))   # [D, T]
    sn_s = sin.T.copy()
    sn_s[: D // 2] *= -1.0                              # sign-folded rot_half
    sn = np.ascontiguousarray(np.tile(sn_s, (1, B)))
    anw = np.asarray(inputs["attn_norm_w"], f4)
    fnw = np.asarray(inputs["ffn_norm_w"], f4)
    wq = np.asarray(inputs["wq"], f4) * anw[:, None]
    wk = np.asarray(inputs["wk"], f4) * anw[:, None]
    wv = np.asarray(inputs["wv"], f4) * anw[:, None]
    # wqkv ships bf16 (kernel keeps it SBUF-resident)
    wo = np.asarray(inputs["wo"], f4)
    wg = np.asarray(inputs["w_gate"], f4) * fnw[:, None]
    wu = np.asarray(inputs["w_up"], f4) * fnw[:, None]
    wd = np.asarray(inputs["w_down"], f4)
    I_full = wg.shape[1]
    I_pad = NCORES * IC
    if I_pad > I_full:
        pad = I_pad - I_full
        wg = np.pad(wg, ((0, 0), (0, pad)))
        wu = np.pad(wu, ((0, 0), (0, pad)))
        wd = np.pad(wd, ((0, pad), (0, 0)))

    # causal masks, transposed layout: keep (j*128 + kt) <= q
    kt_i = np.arange(128)[:, None]
    q_i = np.arange(TQ)[None, :]
    msk = np.stack([(j * 128 + kt_i <= q_i).astype(f4) for j in range(DIAG)])

    def pack_slabs(w):
        # [K*128, ncols] -> [ncols//128, 128, K*128]: one contiguous
        # DRAM block per 128-column output slab
        K = w.shape[0] // 128
        nc_ = w.shape[1] // 128
        return np.ascontiguousarray(
            w.reshape(K, 128, nc_, 128).transpose(2, 1, 0, 3)
            .reshape(nc_, 128, K * 128))

    qd, kvd = NQH * D, D  # per-core q cols, kv cols
    in_maps = []
    for c in range(NCORES):
        wqkv = np.concatenate([
            wq[:, c * qd:(c + 1) * qd],
            wk[:, c * kvd:(c + 1) * kvd],
            wv[:, c * kvd:(c + 1) * kvd]], axis=1)
        wgu = np.concatenate([
            pack_slabs(wg[:, c * IC:(c + 1) * IC]),
            pack_slabs(wu[:, c * IC:(c + 1) * IC])], axis=0).astype(BF16)
        in_maps.append({
            "xt": xt,
            "xs": np.ascontiguousarray(xt[:, c * TS:(c + 1) * TS]),
            "wqkv": np.ascontiguousarray(wqkv.astype(BF16)),
            "wo": pack_slabs(wo[c * qd:(c + 1) * qd, :]).astype(BF16),
            "wgu": np.ascontiguousarray(wgu),
            "wdn": pack_slabs(wd[c * IC:(c + 1) * IC, :]).astype(BF16),
            "cs": cs, "sn": sn, "msk": msk,
        })
    return in_maps


def assemble_output(results, cfg):
    H, S, B, NCORES = cfg["H"], cfg["S"], cfg["B"], cfg["NC"]
    full_t = np.concatenate([r["out_t"] for r in results], axis=1)  # [H, T]
    return np.ascontiguousarray(full_t.T).reshape(B, S, H)


_NC_CACHE = {}


def _get_nc(cfg_key, cfg):
    if cfg_key not in _NC_CACHE:
        _NC_CACHE[cfg_key] = build_nc(cfg)
    return _NC_CACHE[cfg_key]


def run(inputs, cfg, **kwargs):
    nc = _get_nc(tuple(sorted(cfg.items())), cfg)
    in_maps = prepare_inputs(inputs, cfg)
    res = run_bass_kernel_spmd(nc, in_maps,
                               core_ids=list(range(cfg["NC"])), **kwargs)
    return assemble_output(res.results, cfg), res


def kernel(**inputs) -> np.ndarray:
    out, _ = run(inputs, FULL_CFG)
    return out

